# revision 12
# baseline (speedup 1.0000x reference)
"""Trainium2 Bass kernel for nn_DiffusionNCA_fft2 (B=32, S=64, C=32, HID=256).

Self-contained: takes FULL inputs (as from setup_inputs()), shards batch over
8 NeuronCores (4 per core), runs one SPMD Bass program, gathers FULL output.

V2 redesign vs the 517us baseline:
  - batched (quad) T1 bounce: F1 evacs for all 4 batch elems interleave into
    one [128, (v, b4, c)] tile so the mid-FFT transpose gather reads 256B
    units (2048 descs/b) instead of 64B units (8192 descs/b)
  - strided matmul rhs APs make F2/ifftA psum come out directly in the
    layout the next shuffle wants -> all psum evacs are contiguous
  - dx B-half built by one SBUF->SBUF partition-shift DMA instead of
    re-reading HBM (inherits all reflect pads for free)
  - D3 dumped contiguous; T3/T4 read strided at 128B units (pair-batched T4)
  - PE warmup matmul block + big consts moved to the gpsimd DMA ring so the
    first real matmul isn't stuck behind 6MB of constants
  - sq-pass and mask/hw elementwise offloaded to GPSIMD
"""

import os
from contextlib import ExitStack

import numpy as np
import ml_dtypes

import concourse.bass as bass
import concourse.mybir as mybir
import concourse.tile as tile
from concourse import bacc

S = 64
C = 32
C2 = 64
HID = 256
B = 32
NCORES = 8
BPC = B // NCORES            # batch per core
SP = 66                      # padded spatial
NPAD = SP * SP               # 4356
NPIX = S * S                 # 4096
LN_N = float(HID * NPIX)     # LN element count per batch
EPS = 1e-5
FIRE = 0.5

f32 = mybir.dt.float32
bf16 = mybir.dt.bfloat16
AF = mybir.ActivationFunctionType
ALU = mybir.AluOpType

_BF = ml_dtypes.bfloat16


def _dft_mats():
    t = np.arange(S)
    ang = -2.0 * np.pi * np.outer(t, t) / S
    return np.cos(ang).astype(np.float32), np.sin(ang).astype(np.float32)


def host_constants(inp):
    """All per-core constant inputs, in device layouts (shared by all cores)."""
    Fr, Fi = _dft_mats()
    cst = {}

    ff1 = np.zeros((S, 2 * S), np.float32)
    ff1[:, :S], ff1[:, S:] = Fr.T, Fi.T
    cst["ff1"] = ff1.astype(_BF)

    w2 = np.zeros((2 * S, 2 * S), np.float32)
    w2[:S, :S], w2[S:, :S] = Fr.T, -Fi.T
    w2[:S, S:], w2[S:, S:] = Fi.T, Fr.T
    cst["w2"] = w2.astype(_BF)

    Gr, Gi = Fr / S, -Fi / S
    wa = np.zeros((2 * S, 2 * S), np.float32)
    wa[:S, :S], wa[S:, :S] = Gr.T, -Gi.T
    wa[:S, S:], wa[S:, S:] = Gi.T, Gr.T
    cst["wa"] = wa.astype(_BF)

    a = np.linspace(1.0, 0.0, S, dtype=np.float32)
    alive = (a[:, None] + a[None, :]) * 0.5
    cst["alive"] = np.pad(alive, 1, mode="reflect").reshape(-1).astype(_BF)

    p0w, p1w = np.asarray(inp["p0_w"]), np.asarray(inp["p1_w"])
    wpair = np.zeros((2 * C2, 3 * 2 * C2), np.float32)  # [k, di*128 + m]
    wsing = np.zeros((C2, 3 * 2 * C2), np.float32)
    for di in range(3):
        mo = di * 2 * C2
        wpair[:C2, mo:mo + C2] = p0w[:, :, di, 0].T
        wpair[C2:, mo:mo + C2] = p0w[:, :, di, 1].T
        wpair[:C2, mo + C2:mo + 2 * C2] = p1w[:, :, di, 0].T
        wpair[C2:, mo + C2:mo + 2 * C2] = p1w[:, :, di, 1].T
        wsing[:, mo:mo + C2] = p0w[:, :, di, 2].T
        wsing[:, mo + C2:mo + 2 * C2] = p1w[:, :, di, 2].T
    cst["wpair"] = wpair.astype(_BF)
    cst["wsing"] = wsing.astype(_BF)

    fc0w = np.asarray(inp["fc0_w"])
    cst["fc0a"] = fc0w[:C2].astype(_BF)          # [64, 256]
    cst["fc0bb"] = fc0w[C2:].astype(_BF)          # [128, 256]
    fc0b = (np.asarray(inp["fc0_b"])
            + np.asarray(inp["p0_b"]) @ fc0w[C2:2 * C2]
            + np.asarray(inp["p1_b"]) @ fc0w[2 * C2:])
    cst["fc0b2"] = fc0b.reshape(2, 128).T.astype(np.float32).copy()  # [128, 2]

    fc1w = np.asarray(inp["fc1_w"]).astype(np.float32)  # [256, 64]
    fc1t = np.zeros((128, 128), np.float32)
    fc1t[:, :64], fc1t[:, 64:] = fc1w[:128], fc1w[128:]
    cst["fc1"] = fc1t.astype(_BF)

    lnw = np.asarray(inp["ln_w"]).astype(np.float32)
    lnb = np.asarray(inp["ln_b"]).astype(np.float32)
    lnw_dev = np.transpose(lnw, (2, 1, 0)).reshape(HID, NPIX)  # [k, (m,n)]
    lnb_dev = np.transpose(lnb, (2, 1, 0)).reshape(HID, NPIX)
    cst["lnw"] = np.concatenate([lnw_dev[:128], lnw_dev[128:]], axis=1).astype(_BF)
    lw1 = fc1w[:128].T @ lnw_dev[:128] + fc1w[128:].T @ lnw_dev[128:]  # [64, 4096]
    lb1 = fc1w[:128].T @ lnb_dev[:128] + fc1w[128:].T @ lnb_dev[128:]
    cst["lw1t"] = np.concatenate([lw1, lw1], axis=0).astype(_BF)  # [128, 4096]
    cst["lbt"] = np.concatenate([lb1, lb1], axis=0).astype(_BF)
    return cst


def build_nc(steps=1):
    nc = bacc.Bacc("TRN2", target_bir_lowering=False, debug=False)

    # ---- I/O ----
    xs = nc.dram_tensor("xs", [BPC, S, S, C], bf16, kind="ExternalInput")
    ins = {}
    cshape = dict(ff1=([S, 2 * S], bf16), w2=([2 * S, 2 * S], bf16),
                  wa=([2 * S, 2 * S], bf16), alive=([NPAD], bf16),
                  wpair=([2 * C2, 3 * 2 * C2], bf16),
                  wsing=([C2, 3 * 2 * C2], bf16),
                  fc0a=([C2, HID], bf16), fc0bb=([2 * C2, HID], bf16),
                  fc0b2=([128, 2], f32), fc1=([128, 128], bf16),
                  lnw=([128, 2 * NPIX], bf16), lw1t=([128, NPIX], bf16),
                  lbt=([128, NPIX], bf16))
    for name, (shp, dt) in cshape.items():
        ins[name] = nc.dram_tensor(name, shp, dt, kind="ExternalInput")
    maskd = nc.dram_tensor("maskd", [BPC // 2, 128, NPIX], bf16, kind="ExternalInput")

    D1 = nc.dram_tensor("D1", [2, 2 * S, S * 2 * C], bf16)        # [(ri,n),(v,b2,c)]
    D2 = nc.dram_tensor("D2", [BPC, 2 * S, C * S], bf16)           # [(ri,m),(c,n)]
    D3 = nc.dram_tensor("D3", [BPC // 2, 128, NPIX], bf16)         # [(hb,ri,c),(m,n)]
    D4 = nc.dram_tensor("D4", [BPC // 2, 2 * S, S * C * 2], bf16)  # [(ri,j),(n,b2,c)]
    OUT = nc.dram_tensor("OUT", [BPC, 2 * S, S * C], bf16, kind="ExternalOutput")

    with tile.TileContext(nc) as tc, ExitStack() as ctx:
        cpool = ctx.enter_context(tc.tile_pool(name="consts", bufs=1))
        xpool = ctx.enter_context(tc.tile_pool(name="x", bufs=2))
        bigp = ctx.enter_context(tc.tile_pool(name="big", bufs=3))
        xfpool = ctx.enter_context(tc.tile_pool(name="xf", bufs=2))
        dxpool = ctx.enter_context(tc.tile_pool(name="dx", bufs=2))
        ycpool = ctx.enter_context(tc.tile_pool(name="yc", bufs=2))
        hpool = ctx.enter_context(tc.tile_pool(name="h", bufs=4))
        scrp = ctx.enter_context(tc.tile_pool(name="scr", bufs=1))
        spool = ctx.enter_context(tc.tile_pool(name="small", bufs=8))
        zpool = ctx.enter_context(tc.tile_pool(name="z", bufs=1))
        dmpool = ctx.enter_context(tc.tile_pool(name="dm", bufs=2))
        dgpool = ctx.enter_context(tc.tile_pool(name="dg", bufs=2))
        sbpool = ctx.enter_context(tc.tile_pool(name="sb", bufs=2))
        mpool = ctx.enter_context(tc.tile_pool(name="maskp", bufs=1))
        pp = ctx.enter_context(tc.tile_pool(name="pp", bufs=3, space="PSUM"))
        pst = ctx.enter_context(tc.tile_pool(name="pst", bufs=1, space="PSUM"))

        # ---- consts: ff1 first (gates F1), rest after the X loads ----
        ct = {}

        def load_const(name, eng):
            t = cpool.tile(cshape[name][0], cshape[name][1], tag="c_" + name)
            eng.dma_start(t[:], ins[name][:])
            ct[name] = t

        load_const("ff1", nc.sync)

        h_tiles = {}
        stats = {}

        def load_x(b):
            X = xpool.tile([S, S * C], bf16, tag="X", name=f"X_{b}")
            nc.sync.dma_start(X[:], xs[b].rearrange("a b c -> a (b c)"))
            return X

        def late_consts():
            for name in ("w2", "wa", "wpair", "wsing", "fc0a", "fc0bb",
                         "fc0b2", "fc1"):
                load_const(name, nc.sync)
            # big consts on the gpsimd (SWDGE) ring; needed only mid-kernel
            for name in ("lnw", "lw1t", "lbt"):
                load_const(name, nc.gpsimd)
            ones = cpool.tile([128, 128], f32, tag="c_ones")
            nc.gpsimd.memset(ones[:], 1.0)
            ct["ones"] = ones

        def f1(b, X, t1p):
            # psum [(ri,n), (v32,c32)] per half -> strided evac into t1p
            t1pv = t1p[:].rearrange("p (v b c) -> p v b c", v=S, b=2, c=C)
            for half in range(2):
                ps = pp.tile([2 * S, 1024], f32, tag="pp", name=f"f1_{b}_{half}")
                for q in range(2):
                    sl = bass.ds(half * 1024 + q * 512, 512)
                    nc.tensor.matmul(ps[:, bass.ts(q, 512)], ct["ff1"][:], X[:, sl])
                dst = t1pv[:, bass.ts(half, 32), b % 2, :]
                src = ps[:].rearrange("p (v c) -> p v c", v=32, c=C)
                if (b + half) % 2 == 0:
                    nc.vector.tensor_copy(dst, src)
                else:
                    nc.scalar.copy(dst, src)

        def t1_bounce(pi, t1p):
            nc.sync.dma_start(D1[pi][:], t1p[:])
            t1gp = bigp.tile([2 * S, S * 2 * C], bf16, tag="big",
                             name=f"t1gp_{pi}")
            d1v = D1[pi].rearrange("(ri n) (v bc) -> ri v n bc",
                                   ri=2, n=S, v=S, bc=2 * C)
            for ri in range(2):
                eng = nc.sync if ri == 0 else nc.scalar
                eng.dma_start(
                    t1gp[bass.ts(ri, S), :].rearrange("p (n bc) -> p n bc",
                                                      n=S, bc=2 * C),
                    d1v[ri])
            return t1gp

        def f2(b, t1gp):
            # rhs iterates (c, n) per b -> psum free (c,n); contiguous evac
            xf = xfpool.tile([2 * S, C * S], bf16, tag="xf", name=f"xf_{b}")
            rv = t1gp[:].rearrange("p (n bb c) -> p bb c n", n=S, bb=2, c=C)
            for half in range(2):
                ps = pp.tile([2 * S, 1024], f32, tag="pp", name=f"f2_{b}_{half}")
                for q in range(2):
                    nc.tensor.matmul(
                        ps[:, bass.ts(q, 512)], ct["w2"][:],
                        rv[:, b % 2, bass.ds(half * 16 + q * 8, 8), :])
                nc.scalar.copy(xf[:, bass.ts(half, 1024)], ps[:])
            nc.sync.dma_start(D2[b][:], xf[:])

        def build_dx(b):
            # two m-bands so conv T-tiles 0/1 only wait on band 0
            dx2 = dxpool.tile([2 * C2, NPAD], bf16, tag="dx2", name=f"dx_{b}")
            dxv = dx2[:, 0:NPAD].rearrange("q (a b) -> q a b", a=SP, b=SP)
            d2v = D2[b].rearrange("(ri m) (c n) -> ri c m n", ri=2, m=S, c=C, n=S)
            q = slice(0, C2 - 1)
            qb = slice(C2, 2 * C2)
            nc.gpsimd.dma_start(dx2[C2 - 1:C2, 0:NPAD], ins["alive"][None, :])
            for band, (m0, m1) in enumerate(((0, 33), (33, 64))):
                r0, r1 = m0 + 1, m1 + 1      # padded row range
                nc.sync.dma_start(dxv[0:32, r0:r1, 1:S + 1],
                                  d2v[0][:, m0:m1, :])
                # channel 63 is the alive channel: never load its interior
                nc.scalar.dma_start(dxv[32:63, r0:r1, 1:S + 1],
                                    d2v[1][0:31, m0:m1, :])
                nc.vector.tensor_copy(dxv[q, r0:r1, 0:1], dxv[q, r0:r1, 2:3])
                nc.vector.tensor_copy(dxv[q, r0:r1, SP - 1:SP],
                                      dxv[q, r0:r1, SP - 3:SP - 2])
                # B-half: flat +1 shift of this band's rows
                nc.gpsimd.dma_start(dx2[C2:2 * C2, r0 * SP:r1 * SP],
                                    dx2[0:C2, r0 * SP + 1:r1 * SP + 1])
                if band == 0:
                    nc.vector.tensor_copy(dxv[q, 0:1, :], dxv[q, 2:3, :])
                    nc.vector.tensor_copy(dxv[qb, 0:1, 0:S], dxv[qb, 2:3, 0:S])
                else:
                    nc.vector.tensor_copy(dxv[q, SP - 1:SP, :],
                                          dxv[q, SP - 3:SP - 2, :])
                    nc.vector.tensor_copy(dxv[qb, SP - 1:SP, 0:S],
                                          dxv[qb, SP - 3:SP - 2, 0:S])
            return dx2

        def conv_fc0(b, dx2):
            dxv = dx2[:, 0:NPAD].rearrange("q (a b) -> q a b", a=SP, b=SP)
            s1cols = spool.tile([128, 8], f32, tag="s1cols", name=f"s1_{b}")
            s2cols = spool.tile([128, 4], f32, tag="s2cols", name=f"s2_{b}")
            for m in range(2):
                h_tiles[(b, m)] = hpool.tile([128, NPIX], bf16, tag="h",
                                             name=f"h_{b}_{m}")
            for T in range(4):
                r0 = T * 16
                psy = pp.tile([2 * C2, 1024], f32, tag="pp", name=f"psy_{b}_{T}")
                for q in range(2):
                    rq = r0 + q * 8
                    for di in range(3):
                        nc.tensor.matmul(
                            psy[:, bass.ts(q, 512)],
                            ct["wpair"][:, bass.ts(di, 2 * C2)],
                            dxv[:, rq + di:rq + di + 8, 0:S],
                            start=(di == 0), stop=False)
                    for di in range(3):
                        nc.tensor.matmul(
                            psy[:, bass.ts(q, 512)],
                            ct["wsing"][:, bass.ts(di, 2 * C2)],
                            dxv[0:C2, rq + di:rq + di + 8, 2:SP],
                            start=False, stop=(di == 2))
                yc = ycpool.tile([2 * C2, 1024], bf16, tag="yc", name=f"yc_{b}_{T}")
                if T % 2 == 0:
                    nc.vector.tensor_copy(yc[:], psy[:])
                else:
                    nc.scalar.copy(yc[:], psy[:])
                for m in range(2):
                    psh = pp.tile([128, 1024], f32, tag="pp", name=f"psh_{b}_{T}_{m}")
                    for q in range(2):
                        rq = r0 + q * 8
                        nc.tensor.matmul(psh[:, bass.ts(q, 512)],
                                         ct["fc0a"][:, bass.ts(m, 128)],
                                         dxv[0:C2, rq + 1:rq + 9, 1:S + 1],
                                         start=True, stop=False)
                        nc.tensor.matmul(psh[:, bass.ts(q, 512)],
                                         ct["fc0bb"][:, bass.ts(m, 128)],
                                         yc[:, bass.ts(q, 512)],
                                         start=False, stop=True)
                    nc.scalar.activation(
                        h_tiles[(b, m)][:, bass.ts(T, 1024)], psh[:],
                        AF.Prelu, bias=ct["fc0b2"][:, m:m + 1], scale=1.0,
                        alpha=0.01, accum_out=s1cols[:, T * 2 + m:T * 2 + m + 1])
            # sq-pass: m0 chunks on DVE (STT), m1 chunks on ACT (Square)
            scr = scrp.tile([128, 2048], bf16, tag="sqscr", name=f"scr_{b}")
            scr2 = scrp.tile([128, 2048], bf16, tag="sqscr2", name=f"scr2_{b}")
            for t in range(2):
                hs = h_tiles[(b, 0)][:, bass.ts(t, 2048)]
                nc.vector.scalar_tensor_tensor(
                    out=scr[:], in0=hs, scalar=0.0, in1=hs,
                    op0=ALU.bypass, op1=ALU.mult,
                    accum_out=s2cols[:, t:t + 1])
            for t in range(2):
                hs = h_tiles[(b, 1)][:, bass.ts(t, 2048)]
                nc.scalar.activation(
                    scr2[:], hs, AF.Square, bias=0.0, scale=1.0,
                    accum_out=s2cols[:, 2 + t:2 + t + 1])
            stats2 = spool.tile([128, 2], f32, tag="stats2", name=f"stats2_{b}")
            nc.vector.tensor_reduce(stats2[:, 0:1], s1cols[:],
                                    axis=mybir.AxisListType.X, op=ALU.add)
            nc.vector.tensor_reduce(stats2[:, 1:2], s2cols[:],
                                    axis=mybir.AxisListType.X, op=ALU.add)
            psr = pst.tile([128, 2], f32, tag="pst", name=f"pst_{b}")
            nc.tensor.matmul(psr[:], ct["ones"][:], stats2[:])
            mu = spool.tile([128, 1], f32, tag="stat")
            nc.scalar.mul(mu[:], psr[:, 0:1], 1.0 / LN_N)
            msq = spool.tile([128, 1], f32, tag="stat")
            nc.vector.tensor_mul(msq[:], mu[:], mu[:])
            var = spool.tile([128, 1], f32, tag="stat")
            nc.vector.scalar_tensor_tensor(out=var[:], in0=psr[:, 1:2],
                                           scalar=1.0 / LN_N, in1=msq[:],
                                           op0=ALU.mult, op1=ALU.subtract)
            nc.vector.tensor_scalar_add(var[:], var[:], EPS)
            sd = spool.tile([128, 1], f32, tag="stat")
            nc.scalar.activation(sd[:], var[:], AF.Sqrt, bias=0.0, scale=1.0)
            r = spool.tile([128, 1], f32, tag="stat")
            nc.vector.reciprocal(r[:], sd[:])
            nrm = spool.tile([128, 1], f32, tag="stat")
            nc.vector.tensor_mul(nrm[:], r[:], mu[:])
            nc.scalar.mul(nrm[:], nrm[:], -1.0)
            stats[b] = {"r": r, "nrm": nrm}
            # hw-pass in place: h <- h * ln_w  (m0 on DVE, m1 on GPSIMD)
            nc.vector.tensor_mul(h_tiles[(b, 0)][:], h_tiles[(b, 0)][:],
                                 ct["lnw"][:, 0:NPIX])
            nc.vector.tensor_mul(h_tiles[(b, 1)][:], h_tiles[(b, 1)][:],
                                 ct["lnw"][:, NPIX:2 * NPIX])

        def fc1_tail(pair):
            b0, b1 = 2 * pair, 2 * pair + 1
            r2 = spool.tile([128, 1], f32, tag="stat")
            nrm2 = spool.tile([128, 1], f32, tag="stat")
            nc.vector.tensor_copy(r2[0:64, :], stats[b0]["r"][0:64, :])
            nc.vector.tensor_copy(r2[64:128, :], stats[b1]["r"][64:128, :])
            nc.vector.tensor_copy(nrm2[0:64, :], stats[b0]["nrm"][0:64, :])
            nc.vector.tensor_copy(nrm2[64:128, :], stats[b1]["nrm"][64:128, :])
            z = zpool.tile([128, NPIX], bf16, tag="ztile", name=f"z_{pair}")
            nc.vector.scalar_tensor_tensor(
                out=z[:], in0=ct["lw1t"][:], scalar=nrm2[:], in1=ct["lbt"][:],
                op0=ALU.mult, op1=ALU.add)
            mask2 = mpool.tile([128, NPIX], bf16, tag="mask2", name=f"mask2_{pair}")
            nc.gpsimd.dma_start(mask2[:], maskd[pair][:])
            dm = dmpool.tile([128, NPIX], bf16, tag="dm", name=f"dm_{pair}")
            for T in range(4):
                psda = pp.tile([128, 1024], f32, tag="pp", name=f"psda_{pair}_{T}")
                psdb = pp.tile([128, 1024], f32, tag="pp", name=f"psdb_{pair}_{T}")
                for q in range(2):
                    for m in range(2):
                        for half, bb, pt in ((0, b0, psda), (1, b1, psdb)):
                            nc.tensor.matmul(
                                pt[bass.ts(half, 64), bass.ts(q, 512)],
                                ct["fc1"][:, bass.ts(m, 64)],
                                h_tiles[(bb, m)][:, bass.ds(T * 1024 + q * 512, 512)],
                                start=(m == 0), stop=(m == 1),
                                tile_position=(0, half * 64))
                for half, pt in ((0, psda), (1, psdb)):
                    hs = bass.ts(half, 64)
                    nc.vector.scalar_tensor_tensor(
                        out=dm[hs, bass.ts(T, 1024)], in0=pt[hs, :],
                        scalar=r2[hs, :], in1=z[hs, bass.ts(T, 1024)],
                        op0=ALU.mult, op1=ALU.add)
                nc.vector.tensor_mul(dm[:, bass.ts(T, 1024)],
                                     dm[:, bass.ts(T, 1024)],
                                     mask2[:, bass.ts(T, 1024)])
                if T % 2 == 1:
                    nc.sync.dma_start(D3[pair][:, bass.ts(T // 2, 2048)],
                                      dm[:, bass.ts(T // 2, 2048)])

        def ifft_a(pair):
            """T3 read + ifftA for both b of the pair; evacs into sa tile."""
            b0 = 2 * pair
            sa = bigp.tile([2 * S, S * 2 * C], bf16, tag="big",
                           name=f"sa_{pair}")
            sav = sa[:, 0:S * C * 2].rearrange("p (n hb c) -> p n hb c",
                                               n=S, hb=2, c=C)
            d3v = D3[pair].rearrange("(hb ri c) (m n) -> hb ri m c n",
                                     hb=2, ri=2, c=C, m=S, n=S)
            for hb in range(2):
                dg = dgpool.tile([2 * S, C * S], bf16, tag="dg", name=f"dg_{b0 + hb}")
                for ri in range(2):
                    eng = nc.sync if ri == 0 else nc.scalar
                    eng.dma_start(
                        dg[bass.ts(ri, S), :].rearrange("p (c n) -> p c n",
                                                        c=C, n=S),
                        d3v[hb, ri])
                dgv = dg[:].rearrange("p (c n) -> p n c", c=C, n=S)
                for half in range(2):
                    ps = pp.tile([2 * S, 1024], f32, tag="pp",
                                 name=f"ia_{pair}_{hb}_{half}")
                    for q in range(2):
                        nc.tensor.matmul(
                            ps[:, bass.ts(q, 512)], ct["wa"][:],
                            dgv[:, bass.ds(half * 32 + q * 16, 16), :])
                    dst = sav[:, bass.ts(half, 32), hb, :]
                    src = ps[:].rearrange("p (n c) -> p n c", n=32, c=C)
                    if half == 0:
                        nc.vector.tensor_copy(dst, src)
                    else:
                        nc.scalar.copy(dst, src)
            nc.sync.dma_start(
                D4[pair][:], sa[:, 0:S * C * 2])
            return sa

        def ifft_b(pair):
            """T4 read + ifftB + OUT for both b of the pair."""
            b0 = 2 * pair
            sbg = bigp.tile([2 * S, S * 2 * C], bf16, tag="big",
                            name=f"sbg_{pair}")
            d4v = D4[pair].rearrange("(ri j) (n bc) -> ri n j bc",
                                     ri=2, j=S, bc=2 * C)
            for ri in range(2):
                eng = nc.sync if ri == 0 else nc.scalar
                eng.dma_start(
                    sbg[bass.ts(ri, S), 0:S * 2 * C].rearrange(
                        "p (j bc) -> p j bc", j=S, bc=2 * C),
                    d4v[ri])
            sgv = sbg[:, 0:S * 2 * C].rearrange("p (j hb c) -> p hb j c",
                                                j=S, hb=2, c=C)
            for hb in range(2):
                sb = sbpool.tile([2 * S, S * C], bf16, tag="sb",
                                 name=f"sb_{b0 + hb}")
                for half in range(2):
                    ps = pp.tile([2 * S, 1024], f32, tag="pp",
                                 name=f"ib_{pair}_{hb}_{half}")
                    for q in range(2):
                        nc.tensor.matmul(
                            ps[:, bass.ts(q, 512)], ct["wa"][:],
                            sgv[:, hb, bass.ds(half * 32 + q * 16, 16), :])
                    nc.scalar.copy(sb[:, bass.ts(half, 1024)], ps[:])
                nc.sync.dma_start(OUT[b0 + hb][:], sb[:])

        assert steps == 1, "device program built for steps==1"

        # ---- emission: pair-level front-end, then pipelined per-b ----
        Xs = [load_x(b) for b in range(BPC)]
        late_consts()
        t1pA = bigp.tile([2 * S, S * 2 * C], bf16, tag="big", name="t1pA")
        f1(0, Xs[0], t1pA)
        f1(1, Xs[1], t1pA)
        t1gpA = t1_bounce(0, t1pA)
        t1pB = bigp.tile([2 * S, S * 2 * C], bf16, tag="big", name="t1pB")
        f1(2, Xs[2], t1pB)
        f1(3, Xs[3], t1pB)
        f2(0, t1gpA)
        dx0 = build_dx(0)
        f2(1, t1gpA)
        dx1 = build_dx(1)
        t1gpB = t1_bounce(1, t1pB)
        conv_fc0(0, dx0)
        f2(2, t1gpB)
        dx2_ = build_dx(2)
        conv_fc0(1, dx1)
        fc1_tail(0)
        f2(3, t1gpB)
        dx3 = build_dx(3)
        conv_fc0(2, dx2_)
        ifft_a(0)
        ifft_b(0)
        conv_fc0(3, dx3)
        fc1_tail(1)
        ifft_a(1)
        ifft_b(1)

    return nc


_BUILT = {}


def kernel(**inputs):
    x = np.ascontiguousarray(np.asarray(inputs["x"], dtype=np.float32))
    steps = int(np.asarray(inputs["steps"]))
    if steps == 0:
        return x.astype(np.complex64)
    assert steps == 1, f"unsupported steps={steps}"

    cst = host_constants(inputs)
    su = np.asarray(inputs["stoch_u"], dtype=np.float32)[..., 0]   # [B, S, S]
    mask = (su > FIRE).astype(np.float32)
    mask_dev = np.ascontiguousarray(np.transpose(mask, (0, 2, 1))
                                    ).reshape(B, NPIX).astype(_BF)
    mask_pairs = np.empty((B // 2, 128, NPIX), _BF)
    for p in range(B // 2):
        mask_pairs[p, :64] = mask_dev[2 * p][None, :]
        mask_pairs[p, 64:] = mask_dev[2 * p + 1][None, :]

    if "nc" not in _BUILT:
        nc = build_nc(steps=1)
        nc.finalize()
        _BUILT["nc"] = nc
    nc = _BUILT["nc"]

    in_maps = []
    for core in range(NCORES):
        m = {k: np.ascontiguousarray(v) for k, v in cst.items()}
        m["xs"] = x[core * BPC:(core + 1) * BPC].astype(_BF)
        m["maskd"] = mask_pairs[core * (BPC // 2):(core + 1) * (BPC // 2)]
        in_maps.append(m)

    from concourse.bass_utils import run_bass_kernel_spmd
    trace = bool(int(os.environ.get("KERNEL_TRACE", "0")))
    res = run_bass_kernel_spmd(nc, in_maps, list(range(NCORES)), trace=trace)
    if trace and res.exec_time_ns is not None:
        print(f"HW exec time: {res.exec_time_ns} ns")
        if res.instructions_and_trace is not None:
            print("trace:", res.instructions_and_trace[1])

    out = np.empty((B, S, S, C), np.complex64)
    for core in range(NCORES):
        o = np.asarray(res.results[core]["OUT"], dtype=np.float32)  # [BPC,128,2048]
        for j in range(BPC):
            b = core * BPC + j
            re = o[j, :S].reshape(S, S, C)
            im = o[j, S:].reshape(S, S, C)
            out[b] = x[b] + re + 1j * im
    return out


# revision 13
# speedup vs baseline: 1.0027x; 1.0027x over previous
"""Trainium2 Bass kernel for nn_DiffusionNCA_fft2 (B=32, S=64, C=32, HID=256).

Self-contained: takes FULL inputs (as from setup_inputs()), shards batch over
8 NeuronCores (4 per core), runs one SPMD Bass program, gathers FULL output.

V2 redesign vs the 517us baseline:
  - batched (quad) T1 bounce: F1 evacs for all 4 batch elems interleave into
    one [128, (v, b4, c)] tile so the mid-FFT transpose gather reads 256B
    units (2048 descs/b) instead of 64B units (8192 descs/b)
  - strided matmul rhs APs make F2/ifftA psum come out directly in the
    layout the next shuffle wants -> all psum evacs are contiguous
  - dx B-half built by one SBUF->SBUF partition-shift DMA instead of
    re-reading HBM (inherits all reflect pads for free)
  - D3 dumped contiguous; T3/T4 read strided at 128B units (pair-batched T4)
  - PE warmup matmul block + big consts moved to the gpsimd DMA ring so the
    first real matmul isn't stuck behind 6MB of constants
  - sq-pass and mask/hw elementwise offloaded to GPSIMD
"""

import os
from contextlib import ExitStack

import numpy as np
import ml_dtypes

import concourse.bass as bass
import concourse.mybir as mybir
import concourse.tile as tile
from concourse import bacc

S = 64
C = 32
C2 = 64
HID = 256
B = 32
NCORES = 8
BPC = B // NCORES            # batch per core
SP = 66                      # padded spatial
NPAD = SP * SP               # 4356
NPIX = S * S                 # 4096
LN_N = float(HID * NPIX)     # LN element count per batch
EPS = 1e-5
FIRE = 0.5

f32 = mybir.dt.float32
bf16 = mybir.dt.bfloat16
AF = mybir.ActivationFunctionType
ALU = mybir.AluOpType

_BF = ml_dtypes.bfloat16


def _dft_mats():
    t = np.arange(S)
    ang = -2.0 * np.pi * np.outer(t, t) / S
    return np.cos(ang).astype(np.float32), np.sin(ang).astype(np.float32)


def host_constants(inp):
    """All per-core constant inputs, in device layouts (shared by all cores)."""
    Fr, Fi = _dft_mats()
    cst = {}

    ff1 = np.zeros((S, 2 * S), np.float32)
    ff1[:, :S], ff1[:, S:] = Fr.T, Fi.T
    cst["ff1"] = ff1.astype(_BF)

    w2 = np.zeros((2 * S, 2 * S), np.float32)
    w2[:S, :S], w2[S:, :S] = Fr.T, -Fi.T
    w2[:S, S:], w2[S:, S:] = Fi.T, Fr.T
    cst["w2"] = w2.astype(_BF)

    Gr, Gi = Fr / S, -Fi / S
    wa = np.zeros((2 * S, 2 * S), np.float32)
    wa[:S, :S], wa[S:, :S] = Gr.T, -Gi.T
    wa[:S, S:], wa[S:, S:] = Gi.T, Gr.T
    cst["wa"] = wa.astype(_BF)

    a = np.linspace(1.0, 0.0, S, dtype=np.float32)
    alive = (a[:, None] + a[None, :]) * 0.5
    cst["alive"] = np.pad(alive, 1, mode="reflect").reshape(-1).astype(_BF)

    p0w, p1w = np.asarray(inp["p0_w"]), np.asarray(inp["p1_w"])
    wpair = np.zeros((2 * C2, 3 * 2 * C2), np.float32)  # [k, di*128 + m]
    wsing = np.zeros((C2, 3 * 2 * C2), np.float32)
    for di in range(3):
        mo = di * 2 * C2
        wpair[:C2, mo:mo + C2] = p0w[:, :, di, 0].T
        wpair[C2:, mo:mo + C2] = p0w[:, :, di, 1].T
        wpair[:C2, mo + C2:mo + 2 * C2] = p1w[:, :, di, 0].T
        wpair[C2:, mo + C2:mo + 2 * C2] = p1w[:, :, di, 1].T
        wsing[:, mo:mo + C2] = p0w[:, :, di, 2].T
        wsing[:, mo + C2:mo + 2 * C2] = p1w[:, :, di, 2].T
    cst["wpair"] = wpair.astype(_BF)
    cst["wsing"] = wsing.astype(_BF)

    fc0w = np.asarray(inp["fc0_w"])
    cst["fc0a"] = fc0w[:C2].astype(_BF)          # [64, 256]
    cst["fc0bb"] = fc0w[C2:].astype(_BF)          # [128, 256]
    fc0b = (np.asarray(inp["fc0_b"])
            + np.asarray(inp["p0_b"]) @ fc0w[C2:2 * C2]
            + np.asarray(inp["p1_b"]) @ fc0w[2 * C2:])
    cst["fc0b2"] = fc0b.reshape(2, 128).T.astype(np.float32).copy()  # [128, 2]

    fc1w = np.asarray(inp["fc1_w"]).astype(np.float32)  # [256, 64]
    fc1t = np.zeros((128, 128), np.float32)
    fc1t[:, :64], fc1t[:, 64:] = fc1w[:128], fc1w[128:]
    cst["fc1"] = fc1t.astype(_BF)

    lnw = np.asarray(inp["ln_w"]).astype(np.float32)
    lnb = np.asarray(inp["ln_b"]).astype(np.float32)
    lnw_dev = np.transpose(lnw, (2, 1, 0)).reshape(HID, NPIX)  # [k, (m,n)]
    lnb_dev = np.transpose(lnb, (2, 1, 0)).reshape(HID, NPIX)
    cst["lnw"] = np.concatenate([lnw_dev[:128], lnw_dev[128:]], axis=1).astype(_BF)
    lw1 = fc1w[:128].T @ lnw_dev[:128] + fc1w[128:].T @ lnw_dev[128:]  # [64, 4096]
    lb1 = fc1w[:128].T @ lnb_dev[:128] + fc1w[128:].T @ lnb_dev[128:]
    cst["lw1t"] = np.concatenate([lw1, lw1], axis=0).astype(_BF)  # [128, 4096]
    cst["lbt"] = np.concatenate([lb1, lb1], axis=0).astype(_BF)
    return cst


def build_nc(steps=1):
    nc = bacc.Bacc("TRN2", target_bir_lowering=False, debug=False)

    # ---- I/O ----
    xs = nc.dram_tensor("xs", [BPC, S, S, C], bf16, kind="ExternalInput")
    ins = {}
    cshape = dict(ff1=([S, 2 * S], bf16), w2=([2 * S, 2 * S], bf16),
                  wa=([2 * S, 2 * S], bf16), alive=([NPAD], bf16),
                  wpair=([2 * C2, 3 * 2 * C2], bf16),
                  wsing=([C2, 3 * 2 * C2], bf16),
                  fc0a=([C2, HID], bf16), fc0bb=([2 * C2, HID], bf16),
                  fc0b2=([128, 2], f32), fc1=([128, 128], bf16),
                  lnw=([128, 2 * NPIX], bf16), lw1t=([128, NPIX], bf16),
                  lbt=([128, NPIX], bf16))
    for name, (shp, dt) in cshape.items():
        ins[name] = nc.dram_tensor(name, shp, dt, kind="ExternalInput")
    maskd = nc.dram_tensor("maskd", [BPC // 2, 128, NPIX], bf16, kind="ExternalInput")

    D1 = nc.dram_tensor("D1", [2, 2 * S, S * 2 * C], bf16)        # [(ri,n),(v,b2,c)]
    D2 = nc.dram_tensor("D2", [BPC, 2 * S, C * S], bf16)           # [(ri,m),(c,n)]
    D3 = nc.dram_tensor("D3", [BPC // 2, 128, NPIX], bf16)         # [(hb,ri,c),(m,n)]
    D4 = nc.dram_tensor("D4", [BPC // 2, 2 * S, S * C * 2], bf16)  # [(ri,j),(n,b2,c)]
    OUT = nc.dram_tensor("OUT", [BPC, 2 * S, S * C], bf16, kind="ExternalOutput")

    with tile.TileContext(nc) as tc, ExitStack() as ctx:
        cpool = ctx.enter_context(tc.tile_pool(name="consts", bufs=1))
        xpool = ctx.enter_context(tc.tile_pool(name="x", bufs=2))
        bigp = ctx.enter_context(tc.tile_pool(name="big", bufs=3))
        xfpool = ctx.enter_context(tc.tile_pool(name="xf", bufs=2))
        dxpool = ctx.enter_context(tc.tile_pool(name="dx", bufs=2))
        ycpool = ctx.enter_context(tc.tile_pool(name="yc", bufs=2))
        hpool = ctx.enter_context(tc.tile_pool(name="h", bufs=4))
        scrp = ctx.enter_context(tc.tile_pool(name="scr", bufs=1))
        spool = ctx.enter_context(tc.tile_pool(name="small", bufs=8))
        zpool = ctx.enter_context(tc.tile_pool(name="z", bufs=1))
        dmpool = ctx.enter_context(tc.tile_pool(name="dm", bufs=2))
        dgpool = ctx.enter_context(tc.tile_pool(name="dg", bufs=2))
        sbpool = ctx.enter_context(tc.tile_pool(name="sb", bufs=2))
        mpool = ctx.enter_context(tc.tile_pool(name="maskp", bufs=1))
        pp = ctx.enter_context(tc.tile_pool(name="pp", bufs=3, space="PSUM"))
        pst = ctx.enter_context(tc.tile_pool(name="pst", bufs=1, space="PSUM"))

        # ---- consts: ff1 first (gates F1), rest after the X loads ----
        ct = {}

        def load_const(name, eng):
            t = cpool.tile(cshape[name][0], cshape[name][1], tag="c_" + name)
            eng.dma_start(t[:], ins[name][:])
            ct[name] = t

        load_const("ff1", nc.sync)

        h_tiles = {}
        stats = {}

        def load_x(b):
            X = xpool.tile([S, S * C], bf16, tag="X", name=f"X_{b}")
            nc.sync.dma_start(X[:], xs[b].rearrange("a b c -> a (b c)"))
            return X

        def late_consts():
            for name in ("w2", "wa", "wpair", "wsing", "fc0a", "fc0bb",
                         "fc0b2", "fc1"):
                load_const(name, nc.sync)
            # big consts on the gpsimd (SWDGE) ring; needed only mid-kernel
            for name in ("lnw", "lw1t", "lbt"):
                load_const(name, nc.gpsimd)
            ones = cpool.tile([128, 128], f32, tag="c_ones")
            nc.gpsimd.memset(ones[:], 1.0)
            ct["ones"] = ones

        def f1(b, X, t1p):
            # psum [(ri,n), (v32,c32)] per half -> strided evac into t1p
            t1pv = t1p[:].rearrange("p (v b c) -> p v b c", v=S, b=2, c=C)
            for half in range(2):
                ps = pp.tile([2 * S, 1024], f32, tag="pp", name=f"f1_{b}_{half}")
                for q in range(2):
                    sl = bass.ds(half * 1024 + q * 512, 512)
                    nc.tensor.matmul(ps[:, bass.ts(q, 512)], ct["ff1"][:], X[:, sl])
                dst = t1pv[:, bass.ts(half, 32), b % 2, :]
                src = ps[:].rearrange("p (v c) -> p v c", v=32, c=C)
                if (b + half) % 2 == 0:
                    nc.vector.tensor_copy(dst, src)
                else:
                    nc.scalar.copy(dst, src)

        def t1_bounce(pi, t1p):
            nc.sync.dma_start(D1[pi][:], t1p[:])
            t1gp = bigp.tile([2 * S, S * 2 * C], bf16, tag="big",
                             name=f"t1gp_{pi}")
            d1v = D1[pi].rearrange("(ri n) (v bc) -> ri v n bc",
                                   ri=2, n=S, v=S, bc=2 * C)
            for ri in range(2):
                eng = nc.sync if ri == 0 else nc.scalar
                eng.dma_start(
                    t1gp[bass.ts(ri, S), :].rearrange("p (n bc) -> p n bc",
                                                      n=S, bc=2 * C),
                    d1v[ri])
            return t1gp

        def f2(b, t1gp):
            # rhs iterates (c, n) per b -> psum free (c,n); contiguous evac
            xf = xfpool.tile([2 * S, C * S], bf16, tag="xf", name=f"xf_{b}")
            rv = t1gp[:].rearrange("p (n bb c) -> p bb c n", n=S, bb=2, c=C)
            for half in range(2):
                ps = pp.tile([2 * S, 1024], f32, tag="pp", name=f"f2_{b}_{half}")
                for q in range(2):
                    nc.tensor.matmul(
                        ps[:, bass.ts(q, 512)], ct["w2"][:],
                        rv[:, b % 2, bass.ds(half * 16 + q * 8, 8), :])
                nc.scalar.copy(xf[:, bass.ts(half, 1024)], ps[:])
            nc.sync.dma_start(D2[b][:], xf[:])

        def build_dx(b):
            # two m-bands so conv T-tiles 0/1 only wait on band 0
            dx2 = dxpool.tile([2 * C2, NPAD], bf16, tag="dx2", name=f"dx_{b}")
            dxv = dx2[:, 0:NPAD].rearrange("q (a b) -> q a b", a=SP, b=SP)
            d2v = D2[b].rearrange("(ri m) (c n) -> ri c m n", ri=2, m=S, c=C, n=S)
            q = slice(0, C2 - 1)
            qb = slice(C2, 2 * C2)
            nc.sync.dma_start(dx2[C2 - 1:C2, 0:NPAD], ins["alive"][None, :])
            for band, (m0, m1) in enumerate(((0, 33), (33, 64))):
                r0, r1 = m0 + 1, m1 + 1      # padded row range
                nc.sync.dma_start(dxv[0:32, r0:r1, 1:S + 1],
                                  d2v[0][:, m0:m1, :])
                # channel 63 is the alive channel: never load its interior
                nc.scalar.dma_start(dxv[32:63, r0:r1, 1:S + 1],
                                    d2v[1][0:31, m0:m1, :])
                nc.vector.tensor_copy(dxv[q, r0:r1, 0:1], dxv[q, r0:r1, 2:3])
                nc.vector.tensor_copy(dxv[q, r0:r1, SP - 1:SP],
                                      dxv[q, r0:r1, SP - 3:SP - 2])
            for band, (m0, m1) in enumerate(((0, 33), (33, 64))):
                r0, r1 = m0 + 1, m1 + 1
                # B-half: flat +1 shift of this band's rows (sync ring, late)
                nc.sync.dma_start(dx2[C2:2 * C2, r0 * SP:r1 * SP],
                                  dx2[0:C2, r0 * SP + 1:r1 * SP + 1])
                if band == 0:
                    nc.vector.tensor_copy(dxv[q, 0:1, :], dxv[q, 2:3, :])
                    nc.vector.tensor_copy(dxv[qb, 0:1, 0:S], dxv[qb, 2:3, 0:S])
                else:
                    nc.vector.tensor_copy(dxv[q, SP - 1:SP, :],
                                          dxv[q, SP - 3:SP - 2, :])
                    nc.vector.tensor_copy(dxv[qb, SP - 1:SP, 0:S],
                                          dxv[qb, SP - 3:SP - 2, 0:S])
            return dx2

        def conv_fc0(b, dx2):
            dxv = dx2[:, 0:NPAD].rearrange("q (a b) -> q a b", a=SP, b=SP)
            s1cols = spool.tile([128, 8], f32, tag="s1cols", name=f"s1_{b}")
            s2cols = spool.tile([128, 4], f32, tag="s2cols", name=f"s2_{b}")
            for m in range(2):
                h_tiles[(b, m)] = hpool.tile([128, NPIX], bf16, tag="h",
                                             name=f"h_{b}_{m}")
            for T in range(4):
                r0 = T * 16
                psy = pp.tile([2 * C2, 1024], f32, tag="pp", name=f"psy_{b}_{T}")
                for q in range(2):
                    rq = r0 + q * 8
                    for di in range(3):
                        nc.tensor.matmul(
                            psy[:, bass.ts(q, 512)],
                            ct["wpair"][:, bass.ts(di, 2 * C2)],
                            dxv[:, rq + di:rq + di + 8, 0:S],
                            start=(di == 0), stop=False)
                    for di in range(3):
                        nc.tensor.matmul(
                            psy[:, bass.ts(q, 512)],
                            ct["wsing"][:, bass.ts(di, 2 * C2)],
                            dxv[0:C2, rq + di:rq + di + 8, 2:SP],
                            start=False, stop=(di == 2))
                yc = ycpool.tile([2 * C2, 1024], bf16, tag="yc", name=f"yc_{b}_{T}")
                if T % 2 == 0:
                    nc.vector.tensor_copy(yc[:], psy[:])
                else:
                    nc.scalar.copy(yc[:], psy[:])
                for m in range(2):
                    psh = pp.tile([128, 1024], f32, tag="pp", name=f"psh_{b}_{T}_{m}")
                    for q in range(2):
                        rq = r0 + q * 8
                        nc.tensor.matmul(psh[:, bass.ts(q, 512)],
                                         ct["fc0a"][:, bass.ts(m, 128)],
                                         dxv[0:C2, rq + 1:rq + 9, 1:S + 1],
                                         start=True, stop=False)
                        nc.tensor.matmul(psh[:, bass.ts(q, 512)],
                                         ct["fc0bb"][:, bass.ts(m, 128)],
                                         yc[:, bass.ts(q, 512)],
                                         start=False, stop=True)
                    nc.scalar.activation(
                        h_tiles[(b, m)][:, bass.ts(T, 1024)], psh[:],
                        AF.Prelu, bias=ct["fc0b2"][:, m:m + 1], scale=1.0,
                        alpha=0.01, accum_out=s1cols[:, T * 2 + m:T * 2 + m + 1])
            # sq-pass: m0 chunks on DVE (STT), m1 chunks on ACT (Square)
            scr = scrp.tile([128, 2048], bf16, tag="sqscr", name=f"scr_{b}")
            scr2 = scrp.tile([128, 2048], bf16, tag="sqscr2", name=f"scr2_{b}")
            for t in range(2):
                hs = h_tiles[(b, 0)][:, bass.ts(t, 2048)]
                nc.vector.scalar_tensor_tensor(
                    out=scr[:], in0=hs, scalar=0.0, in1=hs,
                    op0=ALU.bypass, op1=ALU.mult,
                    accum_out=s2cols[:, t:t + 1])
            for t in range(2):
                hs = h_tiles[(b, 1)][:, bass.ts(t, 2048)]
                nc.scalar.activation(
                    scr2[:], hs, AF.Square, bias=0.0, scale=1.0,
                    accum_out=s2cols[:, 2 + t:2 + t + 1])
            stats2 = spool.tile([128, 2], f32, tag="stats2", name=f"stats2_{b}")
            nc.vector.tensor_reduce(stats2[:, 0:1], s1cols[:],
                                    axis=mybir.AxisListType.X, op=ALU.add)
            nc.vector.tensor_reduce(stats2[:, 1:2], s2cols[:],
                                    axis=mybir.AxisListType.X, op=ALU.add)
            psr = pst.tile([128, 2], f32, tag="pst", name=f"pst_{b}")
            nc.tensor.matmul(psr[:], ct["ones"][:], stats2[:])
            mu = spool.tile([128, 1], f32, tag="stat")
            nc.scalar.mul(mu[:], psr[:, 0:1], 1.0 / LN_N)
            msq = spool.tile([128, 1], f32, tag="stat")
            nc.vector.tensor_mul(msq[:], mu[:], mu[:])
            var = spool.tile([128, 1], f32, tag="stat")
            nc.vector.scalar_tensor_tensor(out=var[:], in0=psr[:, 1:2],
                                           scalar=1.0 / LN_N, in1=msq[:],
                                           op0=ALU.mult, op1=ALU.subtract)
            nc.vector.tensor_scalar_add(var[:], var[:], EPS)
            sd = spool.tile([128, 1], f32, tag="stat")
            nc.scalar.activation(sd[:], var[:], AF.Sqrt, bias=0.0, scale=1.0)
            r = spool.tile([128, 1], f32, tag="stat")
            nc.vector.reciprocal(r[:], sd[:])
            nrm = spool.tile([128, 1], f32, tag="stat")
            nc.vector.tensor_mul(nrm[:], r[:], mu[:])
            nc.scalar.mul(nrm[:], nrm[:], -1.0)
            stats[b] = {"r": r, "nrm": nrm}
            # hw-pass in place: h <- h * ln_w  (m0 on DVE, m1 on GPSIMD)
            nc.vector.tensor_mul(h_tiles[(b, 0)][:], h_tiles[(b, 0)][:],
                                 ct["lnw"][:, 0:NPIX])
            nc.vector.tensor_mul(h_tiles[(b, 1)][:], h_tiles[(b, 1)][:],
                                 ct["lnw"][:, NPIX:2 * NPIX])

        def fc1_tail(pair):
            b0, b1 = 2 * pair, 2 * pair + 1
            r2 = spool.tile([128, 1], f32, tag="stat")
            nrm2 = spool.tile([128, 1], f32, tag="stat")
            nc.vector.tensor_copy(r2[0:64, :], stats[b0]["r"][0:64, :])
            nc.vector.tensor_copy(r2[64:128, :], stats[b1]["r"][64:128, :])
            nc.vector.tensor_copy(nrm2[0:64, :], stats[b0]["nrm"][0:64, :])
            nc.vector.tensor_copy(nrm2[64:128, :], stats[b1]["nrm"][64:128, :])
            z = zpool.tile([128, NPIX], bf16, tag="ztile", name=f"z_{pair}")
            nc.vector.scalar_tensor_tensor(
                out=z[:], in0=ct["lw1t"][:], scalar=nrm2[:], in1=ct["lbt"][:],
                op0=ALU.mult, op1=ALU.add)
            mask2 = mpool.tile([128, NPIX], bf16, tag="mask2", name=f"mask2_{pair}")
            nc.gpsimd.dma_start(mask2[:], maskd[pair][:])
            dm = dmpool.tile([128, NPIX], bf16, tag="dm", name=f"dm_{pair}")
            for T in range(4):
                psda = pp.tile([128, 1024], f32, tag="pp", name=f"psda_{pair}_{T}")
                psdb = pp.tile([128, 1024], f32, tag="pp", name=f"psdb_{pair}_{T}")
                for q in range(2):
                    for m in range(2):
                        for half, bb, pt in ((0, b0, psda), (1, b1, psdb)):
                            nc.tensor.matmul(
                                pt[bass.ts(half, 64), bass.ts(q, 512)],
                                ct["fc1"][:, bass.ts(m, 64)],
                                h_tiles[(bb, m)][:, bass.ds(T * 1024 + q * 512, 512)],
                                start=(m == 0), stop=(m == 1),
                                tile_position=(0, half * 64))
                for half, pt in ((0, psda), (1, psdb)):
                    hs = bass.ts(half, 64)
                    nc.vector.scalar_tensor_tensor(
                        out=dm[hs, bass.ts(T, 1024)], in0=pt[hs, :],
                        scalar=r2[hs, :], in1=z[hs, bass.ts(T, 1024)],
                        op0=ALU.mult, op1=ALU.add)
                nc.vector.tensor_mul(dm[:, bass.ts(T, 1024)],
                                     dm[:, bass.ts(T, 1024)],
                                     mask2[:, bass.ts(T, 1024)])
                if T % 2 == 1:
                    nc.sync.dma_start(D3[pair][:, bass.ts(T // 2, 2048)],
                                      dm[:, bass.ts(T // 2, 2048)])

        def ifft_a(pair):
            """T3 read + ifftA for both b of the pair; evacs into sa tile."""
            b0 = 2 * pair
            sa = bigp.tile([2 * S, S * 2 * C], bf16, tag="big",
                           name=f"sa_{pair}")
            sav = sa[:, 0:S * C * 2].rearrange("p (n hb c) -> p n hb c",
                                               n=S, hb=2, c=C)
            d3v = D3[pair].rearrange("(hb ri c) (m n) -> hb ri m c n",
                                     hb=2, ri=2, c=C, m=S, n=S)
            for hb in range(2):
                dg = dgpool.tile([2 * S, C * S], bf16, tag="dg", name=f"dg_{b0 + hb}")
                for ri in range(2):
                    eng = nc.sync if ri == 0 else nc.scalar
                    eng.dma_start(
                        dg[bass.ts(ri, S), :].rearrange("p (c n) -> p c n",
                                                        c=C, n=S),
                        d3v[hb, ri])
                dgv = dg[:].rearrange("p (c n) -> p n c", c=C, n=S)
                for half in range(2):
                    ps = pp.tile([2 * S, 1024], f32, tag="pp",
                                 name=f"ia_{pair}_{hb}_{half}")
                    for q in range(2):
                        nc.tensor.matmul(
                            ps[:, bass.ts(q, 512)], ct["wa"][:],
                            dgv[:, bass.ds(half * 32 + q * 16, 16), :])
                    dst = sav[:, bass.ts(half, 32), hb, :]
                    src = ps[:].rearrange("p (n c) -> p n c", n=32, c=C)
                    if half == 0:
                        nc.vector.tensor_copy(dst, src)
                    else:
                        nc.scalar.copy(dst, src)
            nc.sync.dma_start(
                D4[pair][:], sa[:, 0:S * C * 2])
            return sa

        def ifft_b(pair):
            """T4 read + ifftB + OUT for both b of the pair."""
            b0 = 2 * pair
            sbg = bigp.tile([2 * S, S * 2 * C], bf16, tag="big",
                            name=f"sbg_{pair}")
            d4v = D4[pair].rearrange("(ri j) (n bc) -> ri n j bc",
                                     ri=2, j=S, bc=2 * C)
            for ri in range(2):
                eng = nc.sync if ri == 0 else nc.scalar
                eng.dma_start(
                    sbg[bass.ts(ri, S), 0:S * 2 * C].rearrange(
                        "p (j bc) -> p j bc", j=S, bc=2 * C),
                    d4v[ri])
            sgv = sbg[:, 0:S * 2 * C].rearrange("p (j hb c) -> p hb j c",
                                                j=S, hb=2, c=C)
            for hb in range(2):
                sb = sbpool.tile([2 * S, S * C], bf16, tag="sb",
                                 name=f"sb_{b0 + hb}")
                for half in range(2):
                    ps = pp.tile([2 * S, 1024], f32, tag="pp",
                                 name=f"ib_{pair}_{hb}_{half}")
                    for q in range(2):
                        nc.tensor.matmul(
                            ps[:, bass.ts(q, 512)], ct["wa"][:],
                            sgv[:, hb, bass.ds(half * 32 + q * 16, 16), :])
                    nc.scalar.copy(sb[:, bass.ts(half, 1024)], ps[:])
                nc.sync.dma_start(OUT[b0 + hb][:], sb[:])

        assert steps == 1, "device program built for steps==1"

        # ---- emission: pair-level front-end, then pipelined per-b ----
        Xs = [load_x(b) for b in range(BPC)]
        late_consts()
        t1pA = bigp.tile([2 * S, S * 2 * C], bf16, tag="big", name="t1pA")
        f1(0, Xs[0], t1pA)
        f1(1, Xs[1], t1pA)
        t1gpA = t1_bounce(0, t1pA)
        t1pB = bigp.tile([2 * S, S * 2 * C], bf16, tag="big", name="t1pB")
        f1(2, Xs[2], t1pB)
        f1(3, Xs[3], t1pB)
        f2(0, t1gpA)
        dx0 = build_dx(0)
        f2(1, t1gpA)
        dx1 = build_dx(1)
        t1gpB = t1_bounce(1, t1pB)
        conv_fc0(0, dx0)
        f2(2, t1gpB)
        dx2_ = build_dx(2)
        conv_fc0(1, dx1)
        fc1_tail(0)
        f2(3, t1gpB)
        dx3 = build_dx(3)
        conv_fc0(2, dx2_)
        ifft_a(0)
        ifft_b(0)
        conv_fc0(3, dx3)
        fc1_tail(1)
        ifft_a(1)
        ifft_b(1)

    return nc


_BUILT = {}


def kernel(**inputs):
    x = np.ascontiguousarray(np.asarray(inputs["x"], dtype=np.float32))
    steps = int(np.asarray(inputs["steps"]))
    if steps == 0:
        return x.astype(np.complex64)
    assert steps == 1, f"unsupported steps={steps}"

    cst = host_constants(inputs)
    su = np.asarray(inputs["stoch_u"], dtype=np.float32)[..., 0]   # [B, S, S]
    mask = (su > FIRE).astype(np.float32)
    mask_dev = np.ascontiguousarray(np.transpose(mask, (0, 2, 1))
                                    ).reshape(B, NPIX).astype(_BF)
    mask_pairs = np.empty((B // 2, 128, NPIX), _BF)
    for p in range(B // 2):
        mask_pairs[p, :64] = mask_dev[2 * p][None, :]
        mask_pairs[p, 64:] = mask_dev[2 * p + 1][None, :]

    if "nc" not in _BUILT:
        nc = build_nc(steps=1)
        nc.finalize()
        _BUILT["nc"] = nc
    nc = _BUILT["nc"]

    in_maps = []
    for core in range(NCORES):
        m = {k: np.ascontiguousarray(v) for k, v in cst.items()}
        m["xs"] = x[core * BPC:(core + 1) * BPC].astype(_BF)
        m["maskd"] = mask_pairs[core * (BPC // 2):(core + 1) * (BPC // 2)]
        in_maps.append(m)

    from concourse.bass_utils import run_bass_kernel_spmd
    trace = bool(int(os.environ.get("KERNEL_TRACE", "0")))
    res = run_bass_kernel_spmd(nc, in_maps, list(range(NCORES)), trace=trace)
    if trace and res.exec_time_ns is not None:
        print(f"HW exec time: {res.exec_time_ns} ns")
        if res.instructions_and_trace is not None:
            print("trace:", res.instructions_and_trace[1])

    out = np.empty((B, S, S, C), np.complex64)
    for core in range(NCORES):
        o = np.asarray(res.results[core]["OUT"], dtype=np.float32)  # [BPC,128,2048]
        for j in range(BPC):
            b = core * BPC + j
            re = o[j, :S].reshape(S, S, C)
            im = o[j, S:].reshape(S, S, C)
            out[b] = x[b] + re + 1j * im
    return out
